# revision 1
# baseline (speedup 1.0000x reference)
"""Trainium2 Bass kernel for a 2-layer categorical GIN encoder (v2).

Graph: N=100000 nodes, E=1600000 edges, 256-dim features.

    x   = concat_i emb_i[x_cat[:, i]]                  # [N, 256]
    h1  = LN1(relu(relu((x + A x) @ w1a + b1a) @ w1b + b1b))
    out = LN2(relu((h1 + A h1) @ w2a + b2a) @ w2b + b2b)

v2 strategy (8 NeuronCores, SPMD), improving on the v1 per-chunk
indirect-DMA kernel (5.95 ms -> ~2.0 ms):
  * All gathers use gpsimd.dma_gather (one SWDGE instruction for
    thousands of rows) on 4 SWDGE queues round-robin.  Each queue
    drains ~10 ns/descriptor, so 4 queues move 512-B rows at
    ~220 GB/s vs ~55 GB/s for the old one-chunk-per-instruction path.
  * dma_gather indices are int16, so the fp16 z tables (N rows) are
    addressed in 4 chunks of 25000 rows; edges are bucketed by
    (dst tile, src chunk) on the host, each bucket padded to a
    multiple of 128 slots (max over cores, so the SPMD program is
    shape-uniform).  Slot pads gather row 0 and carry dst_rel = -1 so
    the one-hot selection matmul ignores them.
  * Aggregation per 128-slot block: S[e, d] = (dst_rel[e] == d) built
    on DVE in fp16, then agg[t] += S.T @ G on the tensor engine with
    fp32 PSUM accumulation.
  * Everything matmul-adjacent is fp16 (exact for the one-hot /
    integer data, ~3 more mantissa bits than the old bf16 kernel).
  * The MLP runs per 128-node tile: PE transposes, fp16 weight
    matmuls, LayerNorm via bn_stats/bn_aggr; relu/casts ride the
    otherwise-idle scalar (ACT) engine.
"""

import numpy as np

# ---------------------------------------------------------------------------
# Problem constants
# ---------------------------------------------------------------------------
N = 100000        # nodes
E = 1600000       # edges
D = 256           # feature dim (in = hidden = out)
EMB = 64          # per-field embedding dim
V = 1000          # categories per field
NF = 4            # categorical fields
NC = 8            # NeuronCores
P = 128           # partitions
LN_EPS = 1e-5

NSH = N // NC             # nodes per core (12500)
NT = (NSH + P - 1) // P   # node tiles per core (98)
LAST_VALID = NSH - (NT - 1) * P  # valid rows in last tile (84)

NCH = 4                   # src chunks (int16 index limit)
CHR = N // NCH            # rows per chunk (25000)
T_G = 4                   # dst tiles per gather group (layers)
TC_G = 8                  # dst tiles per gather group (embedding phase)
NQ = 4                    # SWDGE queues

# z tables are laid out slice-major in two Shared DRAM tensors so each
# half's AllGather can be issued as soon as its producer rows are written.
# The split row (6250/core, table row 50000) coincides with the chunk-1/2
# boundary so every int16 gather chunk lives in exactly one tensor.
SL0 = NSH // 2                    # rows/core in slice 0 (6250)
SL1 = NSH - SL0                   # rows/core in slice 1 (6250)
GOFF1 = NC * SL0                  # = 2 * CHR = 50000
SL_TRIG = SL0 // P                # producer tile whose write completes slice 0


def _permrow(n):
    """node id -> slice-major z table row"""
    k = n // NSH
    r = n - k * NSH
    return np.where(r < SL0, k * SL0 + r, GOFF1 + k * SL1 + (r - SL0))


def _ceil128(x):
    return max(128, -(-int(x) // 128) * 128)


# ---------------------------------------------------------------------------
# Host-side preprocessing
# ---------------------------------------------------------------------------
def _prep_meta(x_cat, edge_index):
    """Bucket edges by (core, dst tile, src chunk); build per-core gather
    metadata.

    Returns (M, idx_mat, drel_mat, cmeta) where
      M[t][c]      padded slot count for bucket (t, c), uniform over cores
      idx_mat[k]   [128, TOT16] int16 dma_gather indices (16-row wrap, x8)
      drel_mat[k]  [128, DCOLS] fp16 dst_rel per slot (G layout), -1 = pad
      cmeta[k]     [128, C16] int16 embedding-gather indices
    """
    src = np.asarray(edge_index[0], dtype=np.int64)
    dst = np.asarray(edge_index[1], dtype=np.int64)

    k = dst // NSH
    d_loc = dst - k * NSH
    t = d_loc // P
    rel = (d_loc - t * P).astype(np.int64)
    p_src = _permrow(src)
    c = p_src // CHR
    idx16 = (p_src - c * CHR).astype(np.int16)

    seg = ((k * NT + t) * NCH + c)
    order = np.argsort(seg, kind="stable")
    seg_s = seg[order]
    rel_s = rel[order]
    idx16_s = idx16[order]

    counts = np.bincount(seg_s, minlength=NC * NT * NCH)
    starts = np.zeros(NC * NT * NCH, dtype=np.int64)
    np.cumsum(counts[:-1], out=starts[1:])
    j = np.arange(E, dtype=np.int64) - starts[seg_s]

    Lmax = counts.reshape(NC, NT, NCH).max(axis=0)       # [NT, NCH]

    # group-segment layout: per (g, c) the group's tile buckets are
    # concatenated with static per-tile lengths Lmax (max over cores, NOT
    # rounded), then the total is rounded to a multiple of 128.  Blocks
    # straddling a tile boundary get one extra matmul per extra tile.
    NG = -(-NT // T_G)
    nidx_gc = np.zeros((NG, NCH), dtype=np.int64)
    intra_off = np.zeros((NT, NCH), dtype=np.int64)      # slot offset of (t,c)
    for g in range(NG):
        tl = list(range(g * T_G, min((g + 1) * T_G, NT)))
        for cc in range(NCH):
            off = 0
            for tt in tl:
                intra_off[tt, cc] = off
                off += int(Lmax[tt, cc])
            nidx_gc[g, cc] = -(-off // P) * P            # round up to x128
    slot_base = np.zeros((NG, NCH), dtype=np.int64)      # global slot offset
    run = 0
    for g in range(NG):
        for cc in range(NCH):
            slot_base[g, cc] = run
            run += nidx_gc[g, cc]
    TOTSLOT = int(run)
    assert TOTSLOT % 128 == 0
    DCOLS = TOTSLOT // 128
    TOT16 = TOTSLOT // 16

    # global slot of each edge; drel is GROUP-relative (0..T_G*128-1)
    g_of_t = np.arange(NT) // T_G
    k_s = seg_s // (NT * NCH)
    t_s = (seg_s // NCH) % NT
    c_s = seg_s % NCH
    slot_s = slot_base[g_of_t[t_s], c_s] + intra_off[t_s, c_s] + j
    relg_s = rel_s + P * (t_s % T_G)

    idx_mat = np.zeros((NC, P, TOT16), dtype=np.int16)
    drel_mat = np.full((NC, P, DCOLS), -1.0, dtype=np.float16)
    drel_mat[k_s, slot_s % P, slot_s // P] = relg_s.astype(np.float16)
    rep = np.arange(8) * 16
    idx_mat[k_s[None, :], (slot_s % 16)[None, :] + rep[:, None],
            (slot_s // 16)[None, :]] = idx16_s[None, :]

    # ---- embedding-phase metadata ----
    xc = np.asarray(x_cat, dtype=np.int64)
    NGC = -(-NT // TC_G)
    CSLOT = 0
    cslot_base = []
    for g in range(NGC):
        ntl = min(TC_G, NT - g * TC_G)
        cslot_base.append(CSLOT)
        CSLOT += ntl * NF * P
    C16 = CSLOT // 16
    cmeta = np.zeros((NC, P, C16), dtype=np.int16)
    n_loc = np.arange(NSH, dtype=np.int64)
    t_n = n_loc // P
    r_n = n_loc % P
    g_n = t_n // TC_G
    tloc_n = t_n % TC_G
    cb = np.asarray(cslot_base, dtype=np.int64)
    for f in range(NF):
        cslot = cb[g_n] + (tloc_n * NF + f) * P + r_n
        for k2 in range(NC):
            val = (f * V + xc[k2 * NSH:(k2 + 1) * NSH, f]).astype(np.int16)
            cmeta[k2, (cslot % 16)[None, :] + rep[:, None],
                  (cslot // 16)[None, :]] = val[None, :]

    return ([[int(Lmax[tt, cc]) for cc in range(NCH)] for tt in range(NT)],
            idx_mat, drel_mat, cmeta)


# ---------------------------------------------------------------------------
# Device program
# ---------------------------------------------------------------------------
_PROGRAM_CACHE = {}


def _build_program(M, use_biases=False, use_ln_gb=False, num_cores=NC):
    import concourse.bacc as bacc
    import concourse.bass as bass
    import concourse.tile as tile
    from concourse import mybir

    f32 = mybir.dt.float32
    f16 = mybir.dt.float16
    i16 = mybir.dt.int16

    # ---- static layout mirrors _prep_meta (group segments) ----
    NG = -(-NT // T_G)
    nidx_gc = [[0] * NCH for _ in range(NG)]
    intra_off = [[0] * NCH for _ in range(NT)]
    for g in range(NG):
        tl = list(range(g * T_G, min((g + 1) * T_G, NT)))
        for cc in range(NCH):
            off = 0
            for tt in tl:
                intra_off[tt][cc] = off
                off += M[tt][cc]
            nidx_gc[g][cc] = -(-off // P) * P
    slot_base = [[0] * NCH for _ in range(NG)]
    run = 0
    for g in range(NG):
        for cc in range(NCH):
            slot_base[g][cc] = run
            run += nidx_gc[g][cc]
    TOTSLOT = run
    DCOLS = TOTSLOT // 128
    TOT16 = TOTSLOT // 16
    Cmax = max(nidx_gc[g][cc] for g in range(NG) for cc in range(NCH)) // 128

    NGC = -(-NT // TC_G)
    cslot_base = []
    CSLOT = 0
    for g in range(NGC):
        ntl = min(TC_G, NT - g * TC_G)
        cslot_base.append(CSLOT)
        CSLOT += ntl * NF * P
    C16 = CSLOT // 16

    nc = bacc.Bacc("TRN2", target_bir_lowering=False, debug=False,
                   num_devices=num_cores, num_swdge_queues=NQ,
                   dynamic_dma_scratch_size=49152)

    # Tile assigns Pool-DMA completion sems round-robin over 8 DMASW lanes;
    # a lane must always serve the same SWDGE queue.  With queue = counter%4
    # every gather keeps lane (counter%8) <-> queue (counter%4) consistent.
    _gq = [0]

    def next_queue():
        q = _gq[0] % NQ
        _gq[0] += 1
        return q

    # The SWDGE descriptor ring holds 2048 descriptors per queue; keep each
    # gather at half that so the Q7 never stalls mid-generation waiting for
    # its own drain.
    GMAX = 1536

    def issue_gather(G, tab_view, idx_sb, islot0, nslots):
        nparts = -(-nslots // GMAX)
        per = -(-(nslots // nparts) // 128) * 128
        done = 0
        while done < nslots:
            n = min(per, nslots - done)
            nc.gpsimd.dma_gather(
                G[:, done // 128:(done + n) // 128, :], tab_view,
                idx_sb[:, (islot0 + done) // 16:(islot0 + done + n) // 16],
                n, n, D, single_packet=False, queue_num=next_queue())
            done += n

    # ---- external I/O ----
    embT_d = nc.dram_tensor("embT", [NF, EMB, V], f32, kind="ExternalInput")
    w1a_d = nc.dram_tensor("w1a", [D, D], f32, kind="ExternalInput")
    w1b_d = nc.dram_tensor("w1b", [D, D], f32, kind="ExternalInput")
    w2a_d = nc.dram_tensor("w2a", [D, D], f32, kind="ExternalInput")
    w2b_d = nc.dram_tensor("w2b", [D, D], f32, kind="ExternalInput")
    cmeta_d = nc.dram_tensor("cmeta", [P, C16], i16, kind="ExternalInput")
    srcm_d = nc.dram_tensor("src_meta", [P, TOT16], i16, kind="ExternalInput")
    drel_d = nc.dram_tensor("dstrel_meta", [P, DCOLS], f16,
                            kind="ExternalInput")
    iota_d = nc.dram_tensor("iota_row", [P, T_G * P], f16,
                            kind="ExternalInput")
    ident_d = nc.dram_tensor("identity", [P, P], f32, kind="ExternalInput")
    bias_d = None
    if use_biases or use_ln_gb:
        # rows: b1a, b1b, b2a, b2b, ln1_g, ln1_b, ln2_g, ln2_b
        bias_d = nc.dram_tensor("biasrows", [8, D], f32, kind="ExternalInput")
    out_d = nc.dram_tensor("out", [NSH, D], f32, kind="ExternalOutput")

    groups = [list(range(num_cores))]

    from contextlib import ExitStack

    with tile.TileContext(nc) as tc, ExitStack() as ctx:
        singles = ctx.enter_context(tc.tile_pool(name="singles", bufs=1))
        dram = ctx.enter_context(tc.tile_pool(name="dram", bufs=1,
                                              space="DRAM"))
        work_p = ctx.enter_context(tc.tile_pool(name="work", bufs=4))
        stat_p = ctx.enter_context(tc.tile_pool(name="stat", bufs=4))
        ps_tr = ctx.enter_context(tc.tile_pool(name="ps_tr", bufs=2,
                                               space="PSUM"))
        ps_mm = ctx.enter_context(tc.tile_pool(name="ps_mm", bufs=2,
                                               space="PSUM"))

        # ---- internal DRAM tables ----
        t_dram = dram.tile([NF * V, D], f16)
        z_shard = dram.tile([NSH, D], f16)
        z_full_a = dram.tile([GOFF1, D], f16, addr_space="Shared")
        z_full_b = dram.tile([N - GOFF1, D], f16, addr_space="Shared")
        z2_shard = dram.tile([NSH, D], f16)
        z2_full_a = dram.tile([GOFF1, D], f16, addr_space="Shared")
        z2_full_b = dram.tile([N - GOFF1, D], f16, addr_space="Shared")

        # ---- persistent SBUF constants ----
        iota_sb = singles.tile([P, T_G * P], f16)
        nc.sync.dma_start(out=iota_sb[:], in_=iota_d[:])
        ident_sb = singles.tile([P, P], f32)
        nc.sync.dma_start(out=ident_sb[:], in_=ident_d[:])
        eps_sb = singles.tile([P, 1], f32)
        nc.vector.memset(eps_sb[:], LN_EPS)

        def load_w16(dram_t, name):
            tiles = []
            for kk in range(2):
                w_sb = work_p.tile([P, D], f32, tag="wstage")
                nc.sync.dma_start(out=w_sb[:],
                                  in_=dram_t[kk * P:(kk + 1) * P, :])
                w_r = singles.tile([P, D], f16, name=f"{name}r_{kk}")
                nc.vector.tensor_copy(out=w_r[:], in_=w_sb[:])
                tiles.append(w_r)
            return tiles

        w1a_rows = []
        for f in range(NF):
            w1a_r = singles.tile([EMB, D], f32, name=f"w1a_r{f}")
            nc.sync.dma_start(out=w1a_r[:],
                              in_=w1a_d[f * EMB:(f + 1) * EMB, :])
            w1a_rows.append(w1a_r)
        w1b_sb = load_w16(w1b_d, "w1b")
        w2a_sb = load_w16(w2a_d, "w2a")
        w2b_sb = load_w16(w2b_d, "w2b")

        bias_sb = None
        if bias_d is not None:
            bias_tile = singles.tile([P, 8, D], f32)
            for r in range(8):
                nc.sync.dma_start(
                    out=bias_tile[:, r, :],
                    in_=bias_d[r].unsqueeze(0).to_broadcast([P, D]))
            bias_sb = [bias_tile[:, r, :] for r in range(8)]

        # persistent edge metadata (shared by both layers)
        srcm_sb = singles.tile([P, TOT16], i16)
        nc.sync.dma_start(out=srcm_sb[:], in_=srcm_d[:])
        drel_sb = singles.tile([P, DCOLS], f16)
        nc.sync.dma_start(out=drel_sb[:], in_=drel_d[:])

        # =================================================================
        # Phase B: t tables  t[f] = emb_f @ w1a[64f:64f+64, :]   -> t_dram
        # =================================================================
        MT = 125  # 1000 = 8 * 125
        with tc.tile_pool(name="embT_p", bufs=2) as embT_p, \
             tc.tile_pool(name="ps_b", bufs=2, space="PSUM") as ps_b:
            for f in range(NF):
                embT_sb = embT_p.tile([EMB, V], f32, tag="embT")
                nc.sync.dma_start(out=embT_sb[:], in_=embT_d[f])
                for jj in range(V // MT):
                    t_ps = ps_b.tile([MT, D], f32, tag="tps")
                    nc.tensor.matmul(out=t_ps[:],
                                     lhsT=embT_sb[:, jj * MT:(jj + 1) * MT],
                                     rhs=w1a_rows[f][:], start=True, stop=True)
                    t_sb = embT_p.tile([MT, D], f16, tag="tsb")
                    nc.scalar.copy(out=t_sb[:], in_=t_ps[:])
                    nc.sync.dma_start(
                        out=t_dram[f * V + jj * MT:f * V + (jj + 1) * MT, :],
                        in_=t_sb[:])

            # =============================================================
            # Phase C: z shard  z[n] = sum_f t[f*V + x_cat[n, f]] -> z_shard
            # =============================================================
            cmeta_sb = embT_p.tile([P, C16], i16, tag="cm")
            nc.sync.dma_start(out=cmeta_sb[:], in_=cmeta_d[:])
            for g in range(NGC):
                ntl = min(TC_G, NT - g * TC_G)
                nix = ntl * NF * P
                Cg = nix // 128
                Gc = embT_p.tile([P, NF * TC_G, D], f16, tag="Gc")
                issue_gather(Gc, t_dram[:], cmeta_sb, cslot_base[g], nix)
                for tl in range(ntl):
                    t0 = g * TC_G + tl
                    valid = LAST_VALID if t0 == NT - 1 else P
                    t01 = embT_p.tile([P, 2, D], f16, tag="t01")
                    nc.vector.tensor_add(out=t01[:],
                                         in0=Gc[:, tl * NF:tl * NF + 2, :],
                                         in1=Gc[:, tl * NF + 2:tl * NF + 4, :])
                    z_t = embT_p.tile([P, D], f16, tag="z_t")
                    nc.vector.tensor_add(out=z_t[:], in0=t01[:, 0, :],
                                         in1=t01[:, 1, :])
                    nc.sync.dma_start(out=z_shard[t0 * P:t0 * P + valid, :],
                                      in_=z_t[:valid, :])
                if g == SL_TRIG // TC_G:
                    # slice 0 complete -> AllGather it while slice 1 computes
                    nc.gpsimd.collective_compute(
                        "AllGather", mybir.AluOpType.bypass,
                        replica_groups=groups,
                        ins=[z_shard[:SL0, :]], outs=[z_full_a[:]])

        nc.gpsimd.collective_compute(
            "AllGather", mybir.AluOpType.bypass, replica_groups=groups,
            ins=[z_shard[SL0:, :]], outs=[z_full_b[:]])

        # =================================================================
        # Phases D/E: message passing + MLP layers
        # =================================================================
        def mp_layer(layer, gath_p, sel_p, ps_agg):
            tab_ab = (z_full_a, z_full_b) if layer == 1 else (z2_full_a,
                                                               z2_full_b)
            tab_own = z_shard if layer == 1 else z2_shard
            wb_sb = w1b_sb if layer == 1 else w2b_sb
            ba_row, bb_row = (0, 1) if layer == 1 else (2, 3)
            g_row, b_row = (4, 5) if layer == 1 else (6, 7)

            for g in range(NG):
                tl = list(range(g * T_G, min((g + 1) * T_G, NT)))
                ntl = len(tl)
                # per-tile block ranges within each (g, c) segment; a block
                # may straddle two tiles' sub-segments (one matmul per tile)
                ranges = {}
                napp = [0] * ntl
                for cc in range(NCH):
                    for i_t, tt in enumerate(tl):
                        lo = intra_off[tt][cc]
                        hi = lo + M[tt][cc]
                        b_lo, b_hi = lo // P, -(-hi // P)
                        if hi == lo:
                            b_hi = b_lo
                        ranges[(cc, i_t)] = (b_lo, b_hi)
                        napp[i_t] += b_hi - b_lo
                assert all(n > 0 for n in napp)
                seen = [0] * ntl
                # each tile slot padded to 512 f32 = one PSUM bank, so the
                # per-tile accumulation groups may interleave across chunks
                agg_ps = ps_agg.tile([P, T_G, 512], f32, tag="agg")
                for cc in range(NCH):
                    nix = nidx_gc[g][cc]
                    Cg = nix // 128
                    G = gath_p.tile([P, Cmax, D], f16, tag="G")
                    tab = tab_ab[cc // 2][(cc % 2) * CHR:(cc % 2 + 1) * CHR, :]
                    issue_gather(G, tab, srcm_sb, slot_base[g][cc], nix)
                    S = sel_p.tile([P, Cmax + T_G - 1, P], f16, tag="S")
                    dc0 = slot_base[g][cc] // 128
                    pk = 0
                    pk_of = []
                    for i_t in range(ntl):
                        b_lo, b_hi = ranges[(cc, i_t)]
                        w = b_hi - b_lo
                        pk_of.append(pk - b_lo)
                        if w > 0:
                            nc.vector.tensor_tensor(
                                out=S[:, pk:pk + w, :],
                                in0=drel_sb[:, dc0 + b_lo:dc0 + b_hi]
                                .unsqueeze(2).to_broadcast([P, w, P]),
                                in1=iota_sb[:, i_t * P:(i_t + 1) * P]
                                .unsqueeze(1).to_broadcast([P, w, P]),
                                op=mybir.AluOpType.is_equal)
                        pk += w
                    for i_t, tt in enumerate(tl):
                        b_lo, b_hi = ranges[(cc, i_t)]
                        for b in range(b_lo, b_hi):
                            seen[i_t] += 1
                            nc.tensor.matmul(out=agg_ps[:, i_t, :D],
                                             lhsT=S[:, pk_of[i_t] + b, :],
                                             rhs=G[:, b, :],
                                             start=(seen[i_t] == 1),
                                             stop=(seen[i_t] == napp[i_t]))

                # ---- MLP per tile ----
                for i_t, tt in enumerate(tl):
                    valid = LAST_VALID if tt == NT - 1 else P
                    zown = work_p.tile([P, D], f16, tag="zown")
                    if valid < P:
                        nc.vector.memset(zown[:], 0.0)
                    nc.sync.dma_start(out=zown[:valid, :],
                                      in_=tab_own[tt * P:tt * P + valid, :])
                    u = work_p.tile([P, D], f32, tag="u")
                    nc.vector.tensor_add(out=u[:], in0=agg_ps[:, i_t, :D],
                                         in1=zown[:])
                    if use_biases:
                        nc.vector.tensor_add(out=u[:], in0=u[:],
                                             in1=bias_sb[ba_row])
                    # transpose(relu(u)) == relu(transpose(u))
                    uT_ps = ps_tr.tile([P, 2, P], f32, tag="uT_ps")
                    for kk in range(2):
                        nc.tensor.transpose(out=uT_ps[:, kk, :],
                                            in_=u[:, kk * P:(kk + 1) * P],
                                            identity=ident_sb[:])
                    uT = work_p.tile([P, 2, P], f16, tag="uT")
                    nc.scalar.activation(out=uT[:], in_=uT_ps[:],
                                         func=mybir.ActivationFunctionType.Relu)
                    v_ps = ps_mm.tile([P, D], f32, tag="v_ps")
                    for kk in range(2):
                        nc.tensor.matmul(out=v_ps[:],
                                         lhsT=uT[:, kk, :], rhs=wb_sb[kk][:],
                                         start=(kk == 0), stop=(kk == 1))
                    r = work_p.tile([P, D], f32, tag="r")
                    if use_biases:
                        nc.vector.tensor_add(out=r[:], in0=v_ps[:],
                                             in1=bias_sb[bb_row])
                        if layer == 1:
                            nc.vector.tensor_scalar_max(out=r[:], in0=r[:],
                                                        scalar1=0.0)
                    else:
                        if layer == 1:
                            nc.scalar.activation(
                                out=r[:], in_=v_ps[:],
                                func=mybir.ActivationFunctionType.Relu)
                        else:
                            nc.scalar.copy(out=r[:], in_=v_ps[:])
                    # --- LayerNorm ---
                    stats = stat_p.tile([P, 6], f32, tag="stats")
                    nc.vector.bn_stats(out=stats[:], in_=r[:])
                    mv = stat_p.tile([P, 2], f32, tag="mv")
                    nc.vector.bn_aggr(out=mv[:], in_=stats[:])
                    nc.scalar.activation(out=mv[:, 1:2], in_=mv[:, 1:2],
                                         func=mybir.ActivationFunctionType.Sqrt,
                                         bias=eps_sb[:], scale=1.0)
                    nc.vector.reciprocal(out=mv[:, 1:2], in_=mv[:, 1:2])
                    # negmb = -mean * rstd; then h = r * rstd + negmb on ACT
                    negmb = stat_p.tile([P, 1], f32, tag="negmb")
                    nc.vector.tensor_scalar(out=negmb[:], in0=mv[:, 0:1],
                                            scalar1=mv[:, 1:2],
                                            scalar2=-1.0,
                                            op0=mybir.AluOpType.mult,
                                            op1=mybir.AluOpType.mult)
                    h = work_p.tile([P, D], f32, tag="h")
                    nc.scalar.activation(
                        out=h[:], in_=r[:],
                        func=mybir.ActivationFunctionType.Identity,
                        bias=negmb[:], scale=mv[:, 1:2])
                    if use_ln_gb:
                        nc.vector.tensor_mul(out=h[:], in0=h[:],
                                             in1=bias_sb[g_row])
                        nc.vector.tensor_add(out=h[:], in0=h[:],
                                             in1=bias_sb[b_row])

                    if layer == 1:
                        hT_ps = ps_tr.tile([P, 2, P], f32, tag="uT_ps")
                        for kk in range(2):
                            nc.tensor.transpose(out=hT_ps[:, kk, :],
                                                in_=h[:, kk * P:(kk + 1) * P],
                                                identity=ident_sb[:])
                        hT = work_p.tile([P, 2, P], f16, tag="uT")
                        nc.scalar.copy(out=hT[:], in_=hT_ps[:])
                        z2_ps = ps_mm.tile([P, D], f32, tag="v_ps")
                        for kk in range(2):
                            nc.tensor.matmul(out=z2_ps[:],
                                             lhsT=hT[:, kk, :],
                                             rhs=w2a_sb[kk][:],
                                             start=(kk == 0), stop=(kk == 1))
                        z2_sb = work_p.tile([P, D], f16, tag="z2_sb")
                        nc.scalar.copy(out=z2_sb[:], in_=z2_ps[:])
                        nc.sync.dma_start(
                            out=z2_shard[tt * P:tt * P + valid, :],
                            in_=z2_sb[:valid, :])
                    else:
                        nc.sync.dma_start(out=out_d[tt * P:tt * P + valid, :],
                                          in_=h[:valid, :])

                if layer == 1 and g == SL_TRIG // T_G:
                    # z2 slice 0 complete -> AllGather under remaining L1
                    nc.gpsimd.collective_compute(
                        "AllGather", mybir.AluOpType.bypass,
                        replica_groups=groups,
                        ins=[z2_shard[:SL0, :]], outs=[z2_full_a[:]])

        with tc.tile_pool(name="gath", bufs=5) as gath_p, \
             tc.tile_pool(name="sel", bufs=4) as sel_p, \
             tc.tile_pool(name="ps_agg", bufs=1, space="PSUM") as ps_agg:
            mp_layer(1, gath_p, sel_p, ps_agg)
            nc.gpsimd.collective_compute(
                "AllGather", mybir.AluOpType.bypass, replica_groups=groups,
                ins=[z2_shard[SL0:, :]], outs=[z2_full_b[:]])
            mp_layer(2, gath_p, sel_p, ps_agg)

    nc.compile()
    return nc


def get_program(M, **kw):
    key = (tuple(tuple(r) for r in M), tuple(sorted(kw.items())))
    if key not in _PROGRAM_CACHE:
        _PROGRAM_CACHE[key] = _build_program(M, **kw)
    return _PROGRAM_CACHE[key]


# ---------------------------------------------------------------------------
# Entry point
# ---------------------------------------------------------------------------
def kernel_with_results(x_cat, edge_index, emb0, emb1, emb2, emb3,
                        w1a, b1a, w1b, b1b, w2a, b2a, w2b, b2b,
                        ln1_g, ln1_b, ln2_g, ln2_b, trace=False):
    from concourse import bass_utils

    M, idx_mat, drel_mat, cmeta = _prep_meta(x_cat, edge_index)

    f32 = np.float32
    embT = np.stack([np.ascontiguousarray(np.asarray(e, f32).T)
                     for e in (emb0, emb1, emb2, emb3)])
    w1a = np.ascontiguousarray(np.asarray(w1a, f32))
    w1b = np.ascontiguousarray(np.asarray(w1b, f32))
    w2a = np.ascontiguousarray(np.asarray(w2a, f32))
    w2b = np.ascontiguousarray(np.asarray(w2b, f32))

    biases = [np.asarray(b, f32) for b in (b1a, b1b, b2a, b2b)]
    lngb = [np.asarray(b, f32) for b in (ln1_g, ln1_b, ln2_g, ln2_b)]
    use_biases = any(np.any(b != 0.0) for b in biases)
    use_ln_gb = (np.any(lngb[0] != 1.0) or np.any(lngb[1] != 0.0)
                 or np.any(lngb[2] != 1.0) or np.any(lngb[3] != 0.0))

    iota_row = np.broadcast_to(
        np.arange(T_G * P, dtype=np.float16), (P, T_G * P)).copy()
    identity = np.eye(P, dtype=f32)

    nc = get_program(M, use_biases=use_biases, use_ln_gb=use_ln_gb)

    in_maps = []
    for k in range(NC):
        m = {
            "embT": embT,
            "w1a": w1a, "w1b": w1b, "w2a": w2a, "w2b": w2b,
            "cmeta": cmeta[k],
            "src_meta": idx_mat[k],
            "dstrel_meta": drel_mat[k],
            "iota_row": iota_row,
            "identity": identity,
        }
        if use_biases or use_ln_gb:
            m["biasrows"] = np.stack(biases + lngb)
        in_maps.append(m)

    res = bass_utils.run_bass_kernel_spmd(nc, in_maps,
                                          core_ids=list(range(NC)),
                                          trace=trace)
    out = np.concatenate([r["out"] for r in res.results], axis=0)
    return out.astype(np.float32), res


def kernel(**inputs):
    out, _ = kernel_with_results(**inputs)
    return out

